# revision 24
# baseline (speedup 1.0000x reference)
"""Trainium2 Bass kernel for nn_MessagePassingLayer (gnn_message_passing).

Computes, for x:[B,C,N,1] f32, edge_index:[B,N,K] i32, alpha scalar:
    out[b,c,n] = x[b,c,n]*(1+alpha) + sum_k x[b,c,edge_index[b,n,k]]

Sharding: B=8 batch samples, one per NeuronCore (data parallel). Edge
indices are intra-sample so there is no cross-core communication.

Mechanism: the neighbor gather+sum is a dense matmul against the
(host-built) adjacency-count matrix:
    m[c, n] = sum_src A[n, src] * x[src, c]
TensorE computes it with Aᵀ streamed from HBM as fp8 (counts 0..16 are
exact in e4m3) and x as stationary in two fp8 terms (hi + residual) that
accumulate into the same PSUM region, recovering ~bf16 precision.
This avoids SWDGE dma_gather row-gathers entirely (the previous approach
was capped by the 4-queue software-DGE 256B-descriptor floor ~134us).

The Aᵀ stream is spread over 6 DMA streams: qSP + qAct HWDGE, gpsimd
dma_start (SWDGE q0), and dma_gather-as-linear-load (sequential indices,
4KB rows) on SWDGE queues 1-3.

Per-core device program:
  - stream Aᵀ fp8 [4096 src, 4096 dst] in 32 tiles of [128, 2x2048]
  - 256 DoubleRow fp8 matmuls accumulate A@x_hi + A@x_w into 8 PSUM
    banks of [64 ch, 512 dst]
  - DVE: out = x*(1+alpha) + psum per column group (bf16 x, bf16 out)
"""
import os
import sys
import types

import numpy as np

B, C, N, K = 8, 64, 4096, 16
NCORES = 8
P = 128
NPAIR = N // (2 * P)     # 16 contraction pair-blocks (DoubleRow: 256 rows)
NHALF = 2                # dst column halves (pipeline psum banks 0-3 / 4-7)
HCOLS = N // NHALF       # 2048 dst cols per half
GRP = 512                # psum bank free size (f32)
NGRP = HCOLS // GRP      # 4 col groups per half

LAST_EXEC_NS = None


# ---------------------------------------------------------------------------
# axon NTFF profile hook shim (the agent image's antenv lacks axon_hooks)
# ---------------------------------------------------------------------------
def _install_profile_shim():
    if "antenv.axon_hooks" in sys.modules:
        return
    try:
        import antenv

        mod = types.ModuleType("antenv.axon_hooks")
        mod._hook = None
        mod.set_axon_ntff_profile_hook = lambda h: setattr(mod, "_hook", h)
        mod.get_axon_ntff_profile_hook = lambda: mod._hook
        sys.modules["antenv.axon_hooks"] = mod
        antenv.axon_hooks = mod
        from trn_agent_boot.trn_boot import _ntff_profile_via_ctypes

        mod.set_axon_ntff_profile_hook(
            _ntff_profile_via_ctypes("/opt/axon/libaxon_pjrt.so")
        )
    except Exception:
        pass


# ---------------------------------------------------------------------------
# Walrus in this container rejects >1 sync-wait per instruction. Split any
# multi-wait instruction into single-wait NoOps on the same engine.
# ---------------------------------------------------------------------------
def _split_multiwaits(nc, mybir):
    cnt = [0]
    for f in nc.m.functions:
        for bb in f.blocks:
            new_list = []
            for ins in bb.instructions:
                si = ins.sync_info
                if si is not None and si.on_wait and len(si.on_wait) > 1:
                    waits = list(si.on_wait)
                    for w in waits[:-1]:
                        cnt[0] += 1
                        nop = mybir.InstNoOp(name=f"I-waitsplit-{cnt[0]}")
                        nop.engine = ins.engine
                        nop.sync_info = mybir.SyncInfo(on_wait=[w], on_update=[])
                        try:
                            nc.register_instruction(nop, overwrite=True)
                        except Exception:
                            pass
                        new_list.append(nop)
                    ins.sync_info = mybir.SyncInfo(
                        on_wait=[waits[-1]], on_update=list(si.on_update)
                    )
                new_list.append(ins)
            bb.instructions = new_list


# ---------------------------------------------------------------------------
# Legalization splits every InstMatmult into Ldweights+Matmult, reloading
# the stationary even when consecutive matmuls share it. Drop the redundant
# reloads (PE array state persists); preserve any sync via a PE NoOp.
# ---------------------------------------------------------------------------
def _dedup_ldweights(nc, mybir):
    dropped = [0]
    for f in nc.m.functions:
        for bb in f.blocks:
            last_sig = None
            new_list = []
            for ins in bb.instructions:
                if isinstance(ins, mybir.InstLdweights):
                    sig = repr(ins.ins[0])
                    if sig == last_sig:
                        si = ins.sync_info
                        dropped[0] += 1
                        if si is not None and (si.on_wait or si.on_update):
                            nop = mybir.InstNoOp(
                                name=f"I-ldwdedup-{dropped[0]}")
                            nop.engine = ins.engine
                            nop.sync_info = si
                            try:
                                nc.register_instruction(nop, overwrite=True)
                            except Exception:
                                pass
                            new_list.append(nop)
                        continue
                    last_sig = sig
                elif isinstance(ins, mybir.InstMatmult):
                    pass  # uses the loaded array, does not clobber it
                elif getattr(ins, "engine", None) == mybir.EngineType.PE:
                    last_sig = None  # unknown PE instruction: be safe
                new_list.append(ins)
            bb.instructions = new_list
    return dropped[0]


# ---------------------------------------------------------------------------
# Device program
# ---------------------------------------------------------------------------
NAT = int(os.environ.get("KERNEL_NAT", "4"))  # rotating Aᵀ SBUF buffers


def _build_program():
    import concourse.mybir as mybir
    import concourse.tile as tile
    from concourse import bacc

    nc = bacc.Bacc("TRN2", target_bir_lowering=False, debug=False,
                   num_devices=NCORES, num_swdge_queues=4,
                   dynamic_dma_scratch_size=32768)
    # Aᵀ fp8 bytes in super-tile layout [ (h, v, p) , (a, t, n) ]:
    #   row (h*4+v)*128 + p, col a*4096 + t*2048 + j
    #     =  A[2048h+j, 1024v+256a+128t+p]
    at_d = nc.dram_tensor("at", [N // 4, 4 * N], mybir.dt.uint8,
                          kind="ExternalInput")
    # stationary x fp8 bytes: [p, (q, t, s, c)] with s = {hi, w}
    xs_d = nc.dram_tensor("xs", [P, NPAIR * 2 * 2 * C], mybir.dt.uint8,
                          kind="ExternalInput")
    # x channel-major bf16 (the (1+alpha)*x term)
    x_d = nc.dram_tensor("x", [C, N], mybir.dt.bfloat16, kind="ExternalInput")
    alpha_d = nc.dram_tensor("alpha", [P, 1], mybir.dt.float32,
                             kind="ExternalInput")
    # linear-load row indices for dma_gather-as-dma: 0..127 wrapped
    li_d = nc.dram_tensor("li", [P, P // 16], mybir.dt.int16,
                          kind="ExternalInput")
    out_d = nc.dram_tensor("out", [C, N], mybir.dt.bfloat16,
                           kind="ExternalOutput")

    fp8 = mybir.dt.float8e4
    NSUP_TOTAL = 4
    NSUP = NSUP_TOTAL

    with tile.TileContext(nc) as tc:
        with tc.tile_pool(name="sbuf", bufs=1) as pool, \
             tc.tile_pool(name="psum", bufs=1, space="PSUM") as ppool:
            xs_sb = pool.tile([P, NPAIR * 2 * 2 * C], mybir.dt.uint8,
                              tag="xs")
            x_sb = pool.tile([C, N], mybir.dt.bfloat16, tag="x")
            al_sb = pool.tile([P, 1], mybir.dt.float32, tag="al")
            li_sb = pool.tile([P, P // 16], mybir.dt.int16, tag="li")
            o_sb = pool.tile([C, N], mybir.dt.bfloat16, tag="o")
            t_sb = [pool.tile([C, GRP], mybir.dt.float32, tag=f"t{g}",
                              name=f"t{g}") for g in range(NHALF * NGRP)]
            ps = [ppool.tile([P, GRP], mybir.dt.float32, tag=f"ps{g}",
                             name=f"ps{g}") for g in range(NHALF * NGRP)]
            at_sb = [pool.tile([P, 8 * HCOLS], mybir.dt.uint8,
                               tag=f"at{u}", name=f"at{u}")
                     for u in range(2 * NSUP_TOTAL)]

            nc.sync.dma_start(out=li_sb[:], in_=li_d.ap())
            nc.scalar.dma_start(out=xs_sb[:], in_=xs_d.ap())
            nc.sync.dma_start(out=al_sb[:], in_=alpha_d.ap())
            nc.scalar.dma_start(out=x_sb[:], in_=x_d.ap())
            nreg = nc.gpsimd.to_reg(P)

            # All 8 Aᵀ super-tile loads issued up-front into dedicated
            # buffers. Exactly 8 SWDGE ops total => each gets its own
            # DMASW sem lane (8 lanes, round-robin), so queue assignment
            # can never conflict. Plain dma_start on SWDGE q0 runs at
            # ~256 GB/s with ~1us desc-gen; dma_gather linear-loads pay
            # ~5us desc-gen each (serial on gpsimd), so split 4/4 and
            # keep the final tile (u7) on the fast q0.
            def at_dstart(u):
                nc.gpsimd.dma_start(
                    out=at_sb[u][:], in_=at_d.ap()[u * P:(u + 1) * P, :])

            def at_gather(u, qn):
                nc.gpsimd.dma_gather(
                    out_ap=at_sb[u][:].rearrange("p (a e) -> p a e", a=1),
                    in_ap=at_d.ap()[u * P:(u + 1) * P, :],
                    idxs_ap=li_sb[:],
                    num_idxs=P,
                    num_idxs_reg=nreg,
                    elem_size=8 * HCOLS,
                    queue_num=qn,
                    single_packet=True,
                )

            at_dstart(0)
            at_gather(1, 1)
            at_gather(3, 2)
            at_gather(5, 3)
            at_dstart(2)
            at_dstart(4)
            at_gather(6, 1)
            at_dstart(7)

            # stationary views: [128, 2, 128] fp8 per pair q (hi|w columns)
            xs3 = xs_sb[:].rearrange("p (q t sc) -> p q t sc",
                                     q=NPAIR, t=2)

            for h in range(NHALF):
                for v in range(NSUP):
                    at4 = at_sb[h * NSUP + v][:].rearrange(
                        "p (a t n) -> p a t n", a=4, t=2).bitcast(fp8)
                    for a in range(4):
                        q = 4 * v + a
                        lhsT = xs3[:, q, :, :].bitcast(fp8)
                        for g in range(NGRP):
                            gi = h * NGRP + g
                            mv = at4[:, a, :, g * GRP:(g + 1) * GRP]
                            nc.tensor.matmul(
                                ps[gi][:],
                                lhsT,
                                mv,
                                start=(v == 0 and a == 0),
                                stop=(v == NSUP - 1 and a == 3),
                                perf_mode=mybir.MatmulPerfMode.DoubleRow,
                            )
                # half h done: fold w-partitions, add (1+alpha)*x, store
                for g in range(NGRP):
                    gi = h * NGRP + g
                    lo = h * HCOLS + g * GRP
                    nc.vector.scalar_tensor_tensor(
                        out=t_sb[gi][:],
                        in0=x_sb[:, lo:lo + GRP],
                        scalar=al_sb[0:C, 0:1],
                        in1=ps[gi][0:C, :],
                        op0=mybir.AluOpType.mult,
                        op1=mybir.AluOpType.add,
                    )
                    nc.vector.tensor_add(
                        out=o_sb[:, lo:lo + GRP], in0=t_sb[gi][:],
                        in1=ps[gi][C:2 * C, :],
                    )
                nc.scalar.dma_start(
                    out=out_d.ap()[:, h * HCOLS:(h + 1) * HCOLS],
                    in_=o_sb[:, h * HCOLS:(h + 1) * HCOLS],
                )

    nc.compile()
    if bool(int(os.environ.get("KERNEL_DEDUP_LDW", "1"))):
        _dedup_ldweights(nc, mybir)
    _split_multiwaits(nc, mybir)
    return nc


_PROGRAM = None


def _get_program():
    global _PROGRAM
    if _PROGRAM is None:
        _PROGRAM = _build_program()
    return _PROGRAM


# ---------------------------------------------------------------------------
# Host glue
# ---------------------------------------------------------------------------
def _fp8_lut():
    import ml_dtypes

    return np.arange(K + 1).astype(ml_dtypes.float8_e4m3fn).view(np.uint8)


_LUT = None


def _prep_at(edge_b):
    """edge_b [N, K] int32 -> Aᵀ fp8 bytes in the device tile layout."""
    global _LUT
    if _LUT is None:
        _LUT = _fp8_lut()
    src = edge_b.astype(np.int64)                       # [N dst, K]
    flat = (src * N + np.arange(N, dtype=np.int64)[:, None]).ravel()
    cnt = np.bincount(flat, minlength=N * N)            # Aᵀ[src, dst] counts
    at = _LUT[cnt]                                      # uint8 fp8 bytes
    # [src, dst] -> [(h, v, p), (a, t, n)]
    at6 = at.reshape(NPAIR // 4, 4, 2, P, NHALF, HCOLS)  # (v, a, t, p, h, n)
    at6 = at6.transpose(4, 0, 3, 1, 2, 5)               # (h, v, p, a, t, n)
    return np.ascontiguousarray(at6.reshape(N // 4, 4 * N))


def _prep_xs(xt_b):
    """xt_b [N, C] f32 node-major -> stationary fp8 bytes [128, q*t*s*C]."""
    import ml_dtypes

    hi = xt_b.astype(ml_dtypes.float8_e4m3fn)
    w = (xt_b - hi.astype(np.float32)).astype(ml_dtypes.float8_e4m3fn)
    hw = np.stack([hi.view(np.uint8), w.view(np.uint8)], axis=1)  # [N, s, C]
    hw = hw.reshape(NPAIR, 2, P, 2, C)                  # (q, t, p, s, c)
    hw = hw.transpose(2, 0, 1, 3, 4)                    # (p, q, t, s, c)
    return np.ascontiguousarray(hw.reshape(P, NPAIR * 2 * 2 * C))


def _linear_idx():
    """Wrapped int16 indices 0..127 for dma_gather-as-linear-load."""
    w = np.empty((P, P // 16), dtype=np.int16)
    p = np.arange(P) % 16
    for s in range(P // 16):
        w[:, s] = 16 * s + p
    return w


def kernel(x, edge_index, alpha):
    global LAST_EXEC_NS
    _install_profile_shim()
    import ml_dtypes
    from concourse import bass_utils

    x = np.asarray(x)
    edge_index = np.asarray(edge_index)
    alpha_v = np.float32(np.asarray(alpha))

    nc = _get_program()

    li = _linear_idx()
    in_maps = []
    for b in range(B):
        xt = np.ascontiguousarray(x[b, :, :, 0].T)      # [N, C]
        in_maps.append({
            "at": _prep_at(edge_index[b]),
            "xs": _prep_xs(xt),
            "x": x[b, :, :, 0].astype(ml_dtypes.bfloat16),
            "alpha": np.full((P, 1), 1.0 + alpha_v, dtype=np.float32),
            "li": li,
        })

    trace = bool(int(os.environ.get("KERNEL_PROFILE", "0")))
    res = bass_utils.run_bass_kernel_spmd(
        nc, in_maps, core_ids=list(range(NCORES)), trace=trace
    )
    LAST_EXEC_NS = res.exec_time_ns

    out = np.empty((B, C, N, 1), dtype=np.float32)
    for b in range(B):
        out[b, :, :, 0] = res.results[b]["out"].astype(np.float32)
    return out


# revision 26
# speedup vs baseline: 1.2040x; 1.2040x over previous
"""Trainium2 Bass kernel for nn_MessagePassingLayer (gnn_message_passing).

Computes, for x:[B,C,N,1] f32, edge_index:[B,N,K] i32, alpha scalar:
    out[b,c,n] = x[b,c,n]*(1+alpha) + sum_k x[b,c,edge_index[b,n,k]]

Sharding: B=8 batch samples, one per NeuronCore (data parallel). Edge
indices are intra-sample so there is no cross-core communication.

Mechanism: the neighbor gather+sum is a dense matmul against the
(host-built) adjacency-count matrix:
    m[c, n] = sum_src A[n, src] * x[src, c]
TensorE computes it with Aᵀ streamed from HBM as fp8 (counts 0..16 are
exact in e4m3) and x as stationary in two fp8 terms (hi + residual) that
accumulate into the same PSUM region, recovering ~bf16 precision.
This avoids SWDGE dma_gather row-gathers entirely (the previous approach
was capped by the 4-queue software-DGE 256B-descriptor floor ~134us).

The Aᵀ stream is spread over 6 DMA streams: qSP + qAct HWDGE, gpsimd
dma_start (SWDGE q0), and dma_gather-as-linear-load (sequential indices,
4KB rows) on SWDGE queues 1-3.

Per-core device program:
  - stream Aᵀ fp8 [4096 src, 4096 dst] in 32 tiles of [128, 2x2048]
  - 256 DoubleRow fp8 matmuls accumulate A@x_hi + A@x_w into 8 PSUM
    banks of [64 ch, 512 dst]
  - DVE: out = x*(1+alpha) + psum per column group (bf16 x, bf16 out)
"""
import os
import sys
import types

import numpy as np

B, C, N, K = 8, 64, 4096, 16
NCORES = 8
P = 128
NPAIR = N // (2 * P)     # 16 contraction pair-blocks (DoubleRow: 256 rows)
NHALF = 2                # dst column halves (pipeline psum banks 0-3 / 4-7)
HCOLS = N // NHALF       # 2048 dst cols per half
GRP = 512                # psum bank free size (f32)
NGRP = HCOLS // GRP      # 4 col groups per half

LAST_EXEC_NS = None


# ---------------------------------------------------------------------------
# axon NTFF profile hook shim (the agent image's antenv lacks axon_hooks)
# ---------------------------------------------------------------------------
def _install_profile_shim():
    if "antenv.axon_hooks" in sys.modules:
        return
    try:
        import antenv

        mod = types.ModuleType("antenv.axon_hooks")
        mod._hook = None
        mod.set_axon_ntff_profile_hook = lambda h: setattr(mod, "_hook", h)
        mod.get_axon_ntff_profile_hook = lambda: mod._hook
        sys.modules["antenv.axon_hooks"] = mod
        antenv.axon_hooks = mod
        from trn_agent_boot.trn_boot import _ntff_profile_via_ctypes

        mod.set_axon_ntff_profile_hook(
            _ntff_profile_via_ctypes("/opt/axon/libaxon_pjrt.so")
        )
    except Exception:
        pass


# ---------------------------------------------------------------------------
# Walrus in this container rejects >1 sync-wait per instruction. Split any
# multi-wait instruction into single-wait NoOps on the same engine.
# ---------------------------------------------------------------------------
def _split_multiwaits(nc, mybir):
    cnt = [0]
    for f in nc.m.functions:
        for bb in f.blocks:
            new_list = []
            for ins in bb.instructions:
                si = ins.sync_info
                if si is not None and si.on_wait and len(si.on_wait) > 1:
                    waits = list(si.on_wait)
                    for w in waits[:-1]:
                        cnt[0] += 1
                        nop = mybir.InstNoOp(name=f"I-waitsplit-{cnt[0]}")
                        nop.engine = ins.engine
                        nop.sync_info = mybir.SyncInfo(on_wait=[w], on_update=[])
                        try:
                            nc.register_instruction(nop, overwrite=True)
                        except Exception:
                            pass
                        new_list.append(nop)
                    ins.sync_info = mybir.SyncInfo(
                        on_wait=[waits[-1]], on_update=list(si.on_update)
                    )
                new_list.append(ins)
            bb.instructions = new_list


# ---------------------------------------------------------------------------
# Legalization splits every InstMatmult into Ldweights+Matmult, reloading
# the stationary even when consecutive matmuls share it. Drop the redundant
# reloads (PE array state persists); preserve any sync via a PE NoOp.
# ---------------------------------------------------------------------------
def _dedup_ldweights(nc, mybir):
    dropped = [0]
    for f in nc.m.functions:
        for bb in f.blocks:
            last_sig = None
            new_list = []
            for ins in bb.instructions:
                if isinstance(ins, mybir.InstLdweights):
                    sig = repr(ins.ins[0])
                    if sig == last_sig:
                        si = ins.sync_info
                        dropped[0] += 1
                        if si is not None and (si.on_wait or si.on_update):
                            nop = mybir.InstNoOp(
                                name=f"I-ldwdedup-{dropped[0]}")
                            nop.engine = ins.engine
                            nop.sync_info = si
                            try:
                                nc.register_instruction(nop, overwrite=True)
                            except Exception:
                                pass
                            new_list.append(nop)
                        continue
                    last_sig = sig
                elif isinstance(ins, mybir.InstMatmult):
                    pass  # uses the loaded array, does not clobber it
                elif getattr(ins, "engine", None) == mybir.EngineType.PE:
                    last_sig = None  # unknown PE instruction: be safe
                new_list.append(ins)
            bb.instructions = new_list
    return dropped[0]


# ---------------------------------------------------------------------------
# Device program
# ---------------------------------------------------------------------------
NAT = int(os.environ.get("KERNEL_NAT", "4"))  # rotating Aᵀ SBUF buffers


def _build_program():
    import concourse.mybir as mybir
    import concourse.tile as tile
    from concourse import bacc

    nc = bacc.Bacc("TRN2", target_bir_lowering=False, debug=False,
                   num_devices=NCORES, num_swdge_queues=4,
                   dynamic_dma_scratch_size=32768)
    # Aᵀ fp8 bytes in super-tile layout [ (h, v, p) , (a, t, n) ]:
    #   row (h*4+v)*128 + p, col a*4096 + t*2048 + j
    #     =  A[2048h+j, 1024v+256a+128t+p]
    at_d = nc.dram_tensor("at", [N // 4, 4 * N], mybir.dt.uint8,
                          kind="ExternalInput")
    # stationary x fp8 bytes: [p, (q, t, s, c)] with s = {hi, w}
    xs_d = nc.dram_tensor("xs", [P, NPAIR * 2 * 2 * C], mybir.dt.uint8,
                          kind="ExternalInput")
    # x channel-major bf16 (the (1+alpha)*x term)
    x_d = nc.dram_tensor("x", [C, N], mybir.dt.bfloat16, kind="ExternalInput")
    alpha_d = nc.dram_tensor("alpha", [P, 1], mybir.dt.float32,
                             kind="ExternalInput")
    # linear-load row indices for dma_gather-as-dma: 0..127 wrapped
    li_d = nc.dram_tensor("li", [P, P // 16], mybir.dt.int16,
                          kind="ExternalInput")
    out_d = nc.dram_tensor("out", [C, N], mybir.dt.bfloat16,
                           kind="ExternalOutput")

    fp8 = mybir.dt.float8e4
    NSUP_TOTAL = 4
    NSUP = NSUP_TOTAL

    with tile.TileContext(nc) as tc:
        with tc.tile_pool(name="sbuf", bufs=1) as pool, \
             tc.tile_pool(name="psum", bufs=1, space="PSUM") as ppool:
            xs_sb = pool.tile([P, NPAIR * 2 * 2 * C], mybir.dt.uint8,
                              tag="xs")
            x_sb = pool.tile([C, N], mybir.dt.bfloat16, tag="x")
            al_sb = pool.tile([P, 1], mybir.dt.float32, tag="al")
            li_sb = pool.tile([P, P // 16], mybir.dt.int16, tag="li")
            o_sb = pool.tile([C, N], mybir.dt.bfloat16, tag="o")
            t_sb = [pool.tile([C, GRP], mybir.dt.float32, tag=f"t{g}",
                              name=f"t{g}") for g in range(NHALF * NGRP)]
            ps = [ppool.tile([P, GRP], mybir.dt.float32, tag=f"ps{g}",
                             name=f"ps{g}") for g in range(NHALF * NGRP)]
            at_sb = [pool.tile([P, 8 * HCOLS], mybir.dt.uint8,
                               tag=f"at{u}", name=f"at{u}")
                     for u in range(2 * NSUP_TOTAL)]

            nc.sync.dma_start(out=li_sb[:], in_=li_d.ap())
            nc.scalar.dma_start(out=xs_sb[:], in_=xs_d.ap())
            nc.sync.dma_start(out=al_sb[:], in_=alpha_d.ap())
            nc.scalar.dma_start(out=x_sb[:], in_=x_d.ap())
            nreg = nc.gpsimd.to_reg(P)

            # All 8 Aᵀ super-tile loads issued up-front into dedicated
            # buffers. Exactly 8 SWDGE ops total => each gets its own
            # DMASW sem lane (8 lanes, round-robin), so queue assignment
            # can never conflict. Plain dma_start on SWDGE q0 runs at
            # ~256 GB/s with ~1us desc-gen; dma_gather linear-loads pay
            # ~5us desc-gen each (serial on gpsimd), so split 4/4 and
            # keep the final tile (u7) on the fast q0.
            at_load_names = []
            for u in range(8):
                inst = nc.gpsimd.dma_start(
                    out=at_sb[u][:], in_=at_d.ap()[u * P:(u + 1) * P, :])
                at_load_names.append((inst.ins.name, u % 4))

            # stationary views: [128, 2, 128] fp8 per pair q (hi|w columns)
            xs3 = xs_sb[:].rearrange("p (q t sc) -> p q t sc",
                                     q=NPAIR, t=2)

            for h in range(NHALF):
                for v in range(NSUP):
                    at4 = at_sb[h * NSUP + v][:].rearrange(
                        "p (a t n) -> p a t n", a=4, t=2).bitcast(fp8)
                    for a in range(4):
                        q = 4 * v + a
                        lhsT = xs3[:, q, :, :].bitcast(fp8)
                        for g in range(NGRP):
                            gi = h * NGRP + g
                            mv = at4[:, a, :, g * GRP:(g + 1) * GRP]
                            nc.tensor.matmul(
                                ps[gi][:],
                                lhsT,
                                mv,
                                start=(v == 0 and a == 0),
                                stop=(v == NSUP - 1 and a == 3),
                                perf_mode=mybir.MatmulPerfMode.DoubleRow,
                            )
                # half h done: fold w-partitions, add (1+alpha)*x, store
                for g in range(NGRP):
                    gi = h * NGRP + g
                    lo = h * HCOLS + g * GRP
                    nc.vector.scalar_tensor_tensor(
                        out=t_sb[gi][:],
                        in0=x_sb[:, lo:lo + GRP],
                        scalar=al_sb[0:C, 0:1],
                        in1=ps[gi][0:C, :],
                        op0=mybir.AluOpType.mult,
                        op1=mybir.AluOpType.add,
                    )
                    nc.vector.tensor_add(
                        out=o_sb[:, lo:lo + GRP], in0=t_sb[gi][:],
                        in1=ps[gi][C:2 * C, :],
                    )
                nc.scalar.dma_start(
                    out=out_d.ap()[:, h * HCOLS:(h + 1) * HCOLS],
                    in_=o_sb[:, h * HCOLS:(h + 1) * HCOLS],
                )

    nc.compile()
    # Spread the Aᵀ dma_start loads across the 4 SWDGE queue rings. The
    # bass API pins gpsimd dma_start to queue 0 ("qPoolDynamic"); the
    # module declares qPoolDynamic{,1,2,3}, and the ISA routes by the
    # instruction's queue name, so reassign post-compile.
    qmap = dict(at_load_names)
    for f in nc.m.functions:
        for bb in f.blocks:
            for ins in bb.instructions:
                qn = qmap.get(ins.name)
                if qn:
                    ins.queue = f"qPoolDynamic{qn}"
    if bool(int(os.environ.get("KERNEL_DEDUP_LDW", "1"))):
        _dedup_ldweights(nc, mybir)
    _split_multiwaits(nc, mybir)
    return nc


_PROGRAM = None


def _get_program():
    global _PROGRAM
    if _PROGRAM is None:
        _PROGRAM = _build_program()
    return _PROGRAM


# ---------------------------------------------------------------------------
# Host glue
# ---------------------------------------------------------------------------
def _fp8_lut():
    import ml_dtypes

    return np.arange(K + 1).astype(ml_dtypes.float8_e4m3fn).view(np.uint8)


_LUT = None


def _prep_at(edge_b):
    """edge_b [N, K] int32 -> Aᵀ fp8 bytes in the device tile layout."""
    global _LUT
    if _LUT is None:
        _LUT = _fp8_lut()
    src = edge_b.astype(np.int64)                       # [N dst, K]
    flat = (src * N + np.arange(N, dtype=np.int64)[:, None]).ravel()
    cnt = np.bincount(flat, minlength=N * N)            # Aᵀ[src, dst] counts
    at = _LUT[cnt]                                      # uint8 fp8 bytes
    # [src, dst] -> [(h, v, p), (a, t, n)]
    at6 = at.reshape(NPAIR // 4, 4, 2, P, NHALF, HCOLS)  # (v, a, t, p, h, n)
    at6 = at6.transpose(4, 0, 3, 1, 2, 5)               # (h, v, p, a, t, n)
    return np.ascontiguousarray(at6.reshape(N // 4, 4 * N))


def _prep_xs(xt_b):
    """xt_b [N, C] f32 node-major -> stationary fp8 bytes [128, q*t*s*C]."""
    import ml_dtypes

    hi = xt_b.astype(ml_dtypes.float8_e4m3fn)
    w = (xt_b - hi.astype(np.float32)).astype(ml_dtypes.float8_e4m3fn)
    hw = np.stack([hi.view(np.uint8), w.view(np.uint8)], axis=1)  # [N, s, C]
    hw = hw.reshape(NPAIR, 2, P, 2, C)                  # (q, t, p, s, c)
    hw = hw.transpose(2, 0, 1, 3, 4)                    # (p, q, t, s, c)
    return np.ascontiguousarray(hw.reshape(P, NPAIR * 2 * 2 * C))


def _linear_idx():
    """Wrapped int16 indices 0..127 for dma_gather-as-linear-load."""
    w = np.empty((P, P // 16), dtype=np.int16)
    p = np.arange(P) % 16
    for s in range(P // 16):
        w[:, s] = 16 * s + p
    return w


def kernel(x, edge_index, alpha):
    global LAST_EXEC_NS
    _install_profile_shim()
    import ml_dtypes
    from concourse import bass_utils

    x = np.asarray(x)
    edge_index = np.asarray(edge_index)
    alpha_v = np.float32(np.asarray(alpha))

    nc = _get_program()

    li = _linear_idx()
    in_maps = []
    for b in range(B):
        xt = np.ascontiguousarray(x[b, :, :, 0].T)      # [N, C]
        in_maps.append({
            "at": _prep_at(edge_index[b]),
            "xs": _prep_xs(xt),
            "x": x[b, :, :, 0].astype(ml_dtypes.bfloat16),
            "alpha": np.full((P, 1), 1.0 + alpha_v, dtype=np.float32),
            "li": li,
        })

    trace = bool(int(os.environ.get("KERNEL_PROFILE", "0")))
    res = bass_utils.run_bass_kernel_spmd(
        nc, in_maps, core_ids=list(range(NCORES)), trace=trace
    )
    LAST_EXEC_NS = res.exec_time_ns

    out = np.empty((B, C, N, 1), dtype=np.float32)
    for b in range(B):
        out[b, :, :, 0] = res.results[b]["out"].astype(np.float32)
    return out
